# revision 1
# baseline (speedup 1.0000x reference)
"""Causal self-attention Bass kernel for Trainium2, SPMD over 8 NeuronCores.

Problem: B=4, T=2048, C=1024, H=16, HD=64 (fp32).
Sharding: (batch, head-group) — core c handles batch b=c//2 and head group
hg=c%2 (8 heads = 512 features). Each core computes its partial output
projection y_part = O_g @ Wp_g^T; the host sums the two partials per batch
and adds bp.

Matmuls run in float32r (TF32-like, ~1.5e-4 rel err, 4x faster than fp32).

Device layouts (per core):
  xT  [C=1024, T=2048]        x[b] transposed (done host-side)
  Q^T, K^T [512, T]           feature-major; head h -> tile h//2, rows (h%2)*64
  V   16 tiles [128, 8, 65]   token-major, +ones column per head so that
                              P@V_aug also yields the softmax denominator
  logits^T [k, q] blocks      softmax along the PARTITION axis k; no max
                              subtraction (logits bounded); additive key mask
                              enters as the Exp activation's per-partition bias
  O^T [512, T]; y [T, 1024] partial
"""

import numpy as np

B, T, C, H = 4, 2048, 1024, 16
HD = C // H            # 64
N_CORES = 8
GF = 512               # features per head group
HPG = 8                # heads per group
TB = 512               # t-block for projections / q-block width
NTB = T // TB          # 4
NCH = T // 128         # 16 key chunks / token tiles
FB = GF // 128         # 4 feature blocks per group
CCH = C // 128         # 8 contraction chunks

_CACHE = {}


def _build(reps=1):
    import contextlib
    import concourse.bass as bass
    import concourse.tile as tile
    import concourse.mybir as mybir
    from concourse import bacc

    F32 = mybir.dt.float32
    F32R = mybir.dt.float32r
    ID = mybir.ActivationFunctionType.Identity
    EXP = mybir.ActivationFunctionType.Exp

    nc = bacc.Bacc("TRN2", target_bir_lowering=False, debug=False,
                   num_devices=N_CORES)

    xt_d = nc.dram_tensor("xt", [C, T], F32R, kind="ExternalInput")
    wq_d = nc.dram_tensor("wq", [C, GF], F32R, kind="ExternalInput")
    wk_d = nc.dram_tensor("wk", [C, GF], F32R, kind="ExternalInput")
    wv_d = nc.dram_tensor("wv", [C, GF], F32R, kind="ExternalInput")
    wp_d = nc.dram_tensor("wp", [GF, C], F32R, kind="ExternalInput")
    bq_d = nc.dram_tensor("bq", [128, FB], F32, kind="ExternalInput")
    bk_d = nc.dram_tensor("bk", [128, FB], F32, kind="ExternalInput")
    bv_d = nc.dram_tensor("bv", [1, GF], F32R, kind="ExternalInput")
    oner_d = nc.dram_tensor("oner", [1, 128], F32R, kind="ExternalInput")
    one_d = nc.dram_tensor("one", [128, 1], F32R, kind="ExternalInput")
    mb_d = nc.dram_tensor("mb", [128, NCH], F32, kind="ExternalInput")
    tri_d = nc.dram_tensor("tri", [128, 128], F32R, kind="ExternalInput")
    y_d = nc.dram_tensor("y", [T, C], F32, kind="ExternalOutput")

    with tile.TileContext(nc) as tc, contextlib.ExitStack() as ctx:
        # pools alive for the whole kernel (~131 KB/partition)
        sing = ctx.enter_context(tc.tile_pool(name="sing", bufs=1))

        bq_sb = sing.tile([128, FB], F32, name="bq", tag="bq")
        bk_sb = sing.tile([128, FB], F32, name="bk", tag="bk")
        bv_sb = sing.tile([1, GF], F32R, name="bv", tag="bv")
        oner_sb = sing.tile([1, 128], F32R, name="oner", tag="oner")
        mb_sb = sing.tile([128, NCH], F32, name="mb", tag="mb")
        tri_sb = sing.tile([128, 128], F32R, name="tri", tag="tri")
        nc.sync.dma_start(out=bq_sb[:], in_=bq_d[:])
        nc.sync.dma_start(out=bk_sb[:], in_=bk_d[:])
        nc.sync.dma_start(out=bv_sb[:], in_=bv_d[:])
        nc.sync.dma_start(out=oner_sb[:], in_=oner_d[:])
        nc.sync.dma_start(out=mb_sb[:], in_=mb_d[:])
        nc.sync.dma_start(out=tri_sb[:], in_=tri_d[:])

        qt_sb = [sing.tile([128, T], F32R, name=f"qt{i}", tag=f"qt{i}") for i in range(FB)]
        kt_sb = [sing.tile([128, T], F32R, name=f"kt{i}", tag=f"kt{i}") for i in range(FB)]
        ot_sb = [sing.tile([128, T], F32R, name=f"ot{i}", tag=f"ot{i}") for i in range(FB)]
        v_sb = [sing.tile([128, HPG, HD + 1], F32R, name=f"v{i}", tag=f"v{i}") for i in range(NCH)]
        for i in range(NCH):
            nc.sync.dma_start(
                out=v_sb[i][:, :, HD:HD + 1],
                in_=bass.AP(tensor=one_d, offset=0, ap=[[1, 128], [0, HPG], [0, 1]]))

        # ---- phase 1: Q^T, K^T (feature-major) and V (token-major) ----
        def phase1():
         with tc.tile_pool(name="p1w", bufs=1) as p1w, \
             tc.tile_pool(name="xstream", bufs=9) as xstream, \
             tc.tile_pool(name="ps_qk", bufs=2, space="PSUM") as ps_qk, \
             tc.tile_pool(name="ps_v", bufs=2, space="PSUM") as ps_v:
            wq_sb = [p1w.tile([128, GF], F32R, name=f"wq{i}", tag=f"wq{i}") for i in range(CCH)]
            wk_sb = [p1w.tile([128, GF], F32R, name=f"wk{i}", tag=f"wk{i}") for i in range(CCH)]
            wv_sb = [p1w.tile([128, GF], F32R, name=f"wv{i}", tag=f"wv{i}") for i in range(CCH)]
            for i in range(CCH):
                nc.sync.dma_start(out=wq_sb[i][:], in_=wq_d[i * 128:(i + 1) * 128, :])
                nc.sync.dma_start(out=wk_sb[i][:], in_=wk_d[i * 128:(i + 1) * 128, :])
                nc.sync.dma_start(out=wv_sb[i][:], in_=wv_d[i * 128:(i + 1) * 128, :])

            for tb in range(NTB):
                xts = []
                for cc in range(CCH):
                    xt_t = xstream.tile([128, TB], F32R, name="xts", tag="xts")
                    nc.sync.dma_start(
                        out=xt_t[:], in_=xt_d[cc * 128:(cc + 1) * 128,
                                              tb * TB:(tb + 1) * TB])
                    xts.append(xt_t)
                for fb in range(FB):
                    pq = ps_qk.tile([128, TB], F32, name="pq", tag="pq")
                    pk = ps_qk.tile([128, TB], F32, name="pk", tag="pk")
                    for cc in range(CCH):
                        nc.tensor.matmul(pq[:], lhsT=wq_sb[cc][:, fb * 128:(fb + 1) * 128],
                                         rhs=xts[cc][:], start=(cc == 0), stop=(cc == CCH - 1))
                    for cc in range(CCH):
                        nc.tensor.matmul(pk[:], lhsT=wk_sb[cc][:, fb * 128:(fb + 1) * 128],
                                         rhs=xts[cc][:], start=(cc == 0), stop=(cc == CCH - 1))
                    # bias-add copies on DVE so ACT only ever runs Exp (a
                    # function-table swap costs ~2.7us)
                    nc.vector.tensor_scalar_add(qt_sb[fb][:, tb * TB:(tb + 1) * TB],
                                                pq[:], bq_sb[:, fb:fb + 1])
                    nc.vector.tensor_scalar_add(kt_sb[fb][:, tb * TB:(tb + 1) * TB],
                                                pk[:], bk_sb[:, fb:fb + 1])
                for tt in range(4):
                    pv = ps_v.tile([128, GF], F32, name="pv", tag="pv")
                    for cc in range(CCH):
                        nc.tensor.matmul(pv[:], lhsT=xts[cc][:, tt * 128:(tt + 1) * 128],
                                         rhs=wv_sb[cc][:], start=(cc == 0), stop=False)
                    nc.tensor.matmul(pv[:], lhsT=oner_sb[:], rhs=bv_sb[:],
                                     start=False, stop=True)
                    tg = tb * 4 + tt
                    nc.vector.tensor_copy(
                        v_sb[tg][:, :, 0:HD],
                        pv[:].rearrange("p (h d) -> p h d", h=HPG))

        # ---- phases 2+3: attention, then y = O @ Wp_g^T (partial) ----
        def phase23():
         with tc.tile_pool(name="p3w", bufs=1) as p3w, \
             tc.tile_pool(name="pts", bufs=3) as pts, \
             tc.tile_pool(name="misc", bufs=2) as misc, \
             tc.tile_pool(name="rbp", bufs=2) as rbp, \
             tc.tile_pool(name="ystage", bufs=3) as ystage, \
             tc.tile_pool(name="dram", bufs=4, space="DRAM") as dram, \
             tc.tile_pool(name="ps_l", bufs=2, space="PSUM") as ps_l, \
             tc.tile_pool(name="ps_o", bufs=1, space="PSUM") as ps_o, \
             tc.tile_pool(name="ps_y", bufs=2, space="PSUM") as ps_y:
            wp_sb = [p3w.tile([128, C], F32R, name=f"wp{i}", tag=f"wp{i}") for i in range(FB)]
            for i in range(FB):
                nc.sync.dma_start(out=wp_sb[i][:], in_=wp_d[i * 128:(i + 1) * 128, :])

            # qb outer so early token blocks of O^T finish first (lets phase 3
            # start while later q-blocks still run); head PAIRS interleaved —
            # the two K=64 logit matmuls sit at partition offsets 0/64 and run
            # concurrently in the PE array (row tiling), and two independent
            # chunk pipelines hide the L->exp->PV dependency chain.
            for qb in range(NTB):
                n_kc = 4 * qb + 4
                for hp in range(HPG // 2):
                    fb = hp
                    h0, h1 = 2 * hp, 2 * hp + 1
                    pso0 = ps_o.tile([HD + 1, TB], F32, name="pso0", tag="pso0")
                    pso1 = ps_o.tile([HD + 1, TB], F32, name="pso1", tag="pso1")
                    for kc in range(n_kc):
                        j = kc - 4 * qb
                        qoff = max(j, 0) * 128
                        width = TB - qoff
                        ksl = slice(kc * 128, (kc + 1) * 128)
                        qsl = slice(qb * TB + qoff, (qb + 1) * TB)
                        pl0 = ps_l.tile([128, TB], F32, name="pl0", tag="pl0")
                        pl1 = ps_l.tile([128, TB], F32, name="pl1", tag="pl1")
                        nc.tensor.matmul(pl0[:, 0:width], lhsT=kt_sb[fb][0:64, ksl],
                                         rhs=qt_sb[fb][0:64, qsl], start=True, stop=True)
                        nc.tensor.matmul(pl1[:, 0:width], lhsT=kt_sb[fb][64:128, ksl],
                                         rhs=qt_sb[fb][64:128, qsl], start=True, stop=True)
                        pt0 = pts.tile([128, TB], F32R, name="pt0", tag="pt0")
                        pt1 = pts.tile([128, TB], F32R, name="pt1", tag="pt1")
                        nc.scalar.activation(out=pt0[:, 0:width], in_=pl0[:, 0:width],
                                             func=EXP, bias=mb_sb[:, kc:kc + 1],
                                             scale=0.125)
                        nc.scalar.activation(out=pt1[:, 0:width], in_=pl1[:, 0:width],
                                             func=EXP, bias=mb_sb[:, kc:kc + 1],
                                             scale=0.125)
                        if j >= 0:
                            nc.vector.tensor_mul(pt0[:, 0:128], pt0[:, 0:128], tri_sb[:])
                            nc.vector.tensor_mul(pt1[:, 0:128], pt1[:, 0:128], tri_sb[:])
                        nc.tensor.matmul(pso0[:, qoff:TB], lhsT=v_sb[kc][:, h0, :],
                                         rhs=pt0[:, 0:width],
                                         start=(kc == 0), stop=(kc == n_kc - 1),
                                         skip_group_check=True)
                        nc.tensor.matmul(pso1[:, qoff:TB], lhsT=v_sb[kc][:, h1, :],
                                         rhs=pt1[:, 0:width],
                                         start=(kc == 0), stop=(kc == n_kc - 1),
                                         skip_group_check=True)
                    # normalize rows 0:64 by reciprocal of row 64 (denominator);
                    # [1,TB] -> [64,TB] broadcast has to bounce via DRAM (SBUF
                    # APs need a nonzero partition step)
                    for po, pso in ((0, pso0), (64, pso1)):
                        # one fast copy releases the PSUM accumulation bank;
                        # the slow reciprocal/broadcast chain then runs off
                        # the SBUF copy without stalling the next head pair
                        ou = misc.tile([HD + 1, TB], F32, name="ou", tag="ou")
                        nc.vector.tensor_copy(ou[:], pso[:])
                        r = misc.tile([1, TB], F32, name="r", tag="r")
                        nc.vector.reciprocal(r[:], ou[HD:HD + 1, :])
                        rd = dram.tile([1, TB], F32, name="rd", tag="rd")
                        nc.sync.dma_start(out=rd[:], in_=r[:])
                        rb = rbp.tile([64, TB], F32, name="rb", tag="rb")
                        nc.sync.dma_start(
                            out=rb[:], in_=bass.AP(tensor=rd.tensor, offset=rd.offset,
                                                   ap=[[0, 64], [1, TB]]))
                        nc.vector.tensor_mul(ot_sb[fb][po:po + 64, qb * TB:(qb + 1) * TB],
                                             ou[0:HD, :], rb[:])

            for tt in range(NCH):
                for eb in range(2):
                    py = ps_y.tile([128, TB], F32, name="py", tag="py")
                    for fc in range(FB):
                        nc.tensor.matmul(py[:], lhsT=ot_sb[fc][:, tt * 128:(tt + 1) * 128],
                                         rhs=wp_sb[fc][:, eb * TB:(eb + 1) * TB],
                                         start=(fc == 0), stop=(fc == FB - 1))
                    ys = ystage.tile([128, TB], F32, name="ys", tag="ys")
                    nc.vector.tensor_copy(ys[:], py[:])
                    nc.sync.dma_start(out=y_d[tt * 128:(tt + 1) * 128,
                                                eb * TB:(eb + 1) * TB], in_=ys[:])

        for _rep in range(reps):
            phase1()
            phase23()

    nc.compile()
    return nc


def _get_nc():
    if "nc" not in _CACHE:
        _CACHE["nc"] = _build()
    return _CACHE["nc"]


def make_in_maps(x, Wq, bq, Wk, bk, Wv, bv, Wp, bp, attention_mask):
    x = np.asarray(x, np.float32)
    Wq = np.asarray(Wq, np.float32)
    Wk = np.asarray(Wk, np.float32)
    Wv = np.asarray(Wv, np.float32)
    Wp = np.asarray(Wp, np.float32)
    bq = np.asarray(bq, np.float32)
    bk = np.asarray(bk, np.float32)
    bv = np.asarray(bv, np.float32)
    mask = np.asarray(attention_mask)

    tri = np.triu(np.ones((128, 128), np.float32))  # tri[k, q] = 1 if k <= q
    oner = np.ones((1, 128), np.float32)
    onec = np.ones((128, 1), np.float32)

    in_maps = []
    for c in range(N_CORES):
        b, hg = divmod(c, 2)
        fsl = slice(hg * GF, (hg + 1) * GF)
        mb = np.where(mask[b] == 0, np.float32(-1e9), np.float32(0.0))
        in_maps.append({
            "xt": np.ascontiguousarray(x[b].T),
            "wq": np.ascontiguousarray(Wq[fsl, :].T),
            "wk": np.ascontiguousarray(Wk[fsl, :].T),
            "wv": np.ascontiguousarray(Wv[fsl, :].T),
            "wp": np.ascontiguousarray(Wp[:, fsl].T),
            "bq": np.ascontiguousarray(bq[fsl].reshape(FB, 128).T),
            "bk": np.ascontiguousarray(bk[fsl].reshape(FB, 128).T),
            "bv": np.ascontiguousarray(bv[fsl].reshape(1, GF)),
            "oner": oner,
            "one": onec,
            "mb": np.ascontiguousarray(mb.reshape(NCH, 128).T.astype(np.float32)),
            "tri": tri,
        })
    return in_maps


def combine(results, bp):
    bp = np.asarray(bp, np.float32)
    y = np.empty((B, T, C), np.float32)
    for b in range(B):
        y[b] = results[2 * b]["y"] + results[2 * b + 1]["y"] + bp[None, :]
    return y


def kernel(x, Wq, bq, Wk, bk, Wv, bv, Wp, bp, attention_mask):
    from concourse.bass_utils import run_bass_kernel_spmd
    nc = _get_nc()
    in_maps = make_in_maps(x, Wq, bq, Wk, bk, Wv, bv, Wp, bp, attention_mask)
    res = run_bass_kernel_spmd(nc, in_maps, list(range(N_CORES)))
    return combine(res.results, bp)



# revision 4
# speedup vs baseline: 11.8684x; 11.8684x over previous
"""Causal self-attention Bass kernel for Trainium2, SPMD over 8 NeuronCores.

Problem: B=4, T=2048, C=1024, H=16, HD=64 (fp32 in/out).
Sharding: (batch, head-group) — core c handles batch b=c//2 and head group
hg=c%2 (8 heads = 512 features). Each core computes its partial output
projection y_part = O_g @ Wp_g^T; the host sums the two partials per batch
and adds bp.

v2: software-pipelined phases + bf16 operands.
  - All matmul operands are bf16 (PSUM accumulation stays fp32), halving
    DMA traffic, SBUF footprint and enabling FWL weight loads.
  - Emission is tb/qb-major: QKV(tb) -> attention(qb=tb) -> proj(qb-1)
    interleaved, so PE fills the gaps of the ACT(exp)-bound attention
    stretches with projection matmuls (engines execute in program order).
  - qt/kt/ot live as per-(fb, tb) tiles so the Tile dependency tracker
    never serializes a consumer on a whole [128, T] tensor.
  - Weights stream on the Activation HWDGE ring, x/y on the SP ring; the
    first QK matmul only waits for wq+xt (~4 us instead of ~35 us).
  - Softmax denominator broadcast [1,TB]->[64,TB] runs on the idle
    GpSimd/Pool engine (partition_broadcast) instead of a DRAM DMA bounce.
  - logits^T layout [k, q]: softmax along the PARTITION axis k; no max
    subtraction (logits bounded); key padding mask enters as the Exp
    activation's per-partition bias; causal diagonal via tri mask multiply.
  - V tiles carry a ones column so P@V_aug also yields the denominator.
"""

import numpy as np

B, T, C, H = 4, 2048, 1024, 16
HD = C // H            # 64
N_CORES = 8
GF = 512               # features per head group
HPG = 8                # heads per group
TB = 512               # t-block width
NTB = T // TB          # 4
NCH = T // 128         # 16 key chunks / token tiles
FB = GF // 128         # 4 feature blocks per group
CCH = C // 128         # 8 contraction chunks

_CACHE = {}


def _build(reps=1):
    import contextlib
    import concourse.bass as bass
    import concourse.tile as tile
    import concourse.mybir as mybir
    from concourse import bacc

    F32 = mybir.dt.float32
    BF16 = mybir.dt.bfloat16
    EXP = mybir.ActivationFunctionType.Exp

    nc = bacc.Bacc("TRN2", target_bir_lowering=False, debug=False,
                   num_devices=N_CORES)

    xt_d = nc.dram_tensor("xt", [C, T], BF16, kind="ExternalInput")
    wq_d = nc.dram_tensor("wq", [C, GF], BF16, kind="ExternalInput")
    wk_d = nc.dram_tensor("wk", [C, GF], BF16, kind="ExternalInput")
    wv_d = nc.dram_tensor("wv", [C, GF], BF16, kind="ExternalInput")
    wp_d = nc.dram_tensor("wp", [GF, C], BF16, kind="ExternalInput")
    bq_d = nc.dram_tensor("bq", [128, FB], F32, kind="ExternalInput")
    bk_d = nc.dram_tensor("bk", [128, FB], F32, kind="ExternalInput")
    bv_d = nc.dram_tensor("bv", [1, GF], BF16, kind="ExternalInput")
    oner_d = nc.dram_tensor("oner", [1, 128], BF16, kind="ExternalInput")
    mb_d = nc.dram_tensor("mb", [128, NCH], F32, kind="ExternalInput")
    tri_d = nc.dram_tensor("tri", [128, 128], BF16, kind="ExternalInput")
    y_d = nc.dram_tensor("y", [T, C], F32, kind="ExternalOutput")

    with tile.TileContext(nc) as tc, contextlib.ExitStack() as ctx:
        sing = ctx.enter_context(tc.tile_pool(name="sing", bufs=1))
        xstream = ctx.enter_context(tc.tile_pool(name="xstream", bufs=10))
        pts = ctx.enter_context(tc.tile_pool(name="pts", bufs=3))
        misc = ctx.enter_context(tc.tile_pool(name="misc", bufs=2))
        rbp = ctx.enter_context(tc.tile_pool(name="rbp", bufs=2))
        ystage = ctx.enter_context(tc.tile_pool(name="ystage", bufs=3))
        ps_a = ctx.enter_context(tc.tile_pool(name="ps_a", bufs=2, space="PSUM"))
        ps_l = ctx.enter_context(tc.tile_pool(name="ps_l", bufs=2, space="PSUM"))
        ps_o = ctx.enter_context(tc.tile_pool(name="ps_o", bufs=1, space="PSUM"))

        # constants + weights on the ACT HWDGE ring; x/y on the SP ring
        bq_sb = sing.tile([128, FB], F32, name="bq", tag="bq")
        bk_sb = sing.tile([128, FB], F32, name="bk", tag="bk")
        bv_sb = sing.tile([1, GF], BF16, name="bv", tag="bv")
        oner_sb = sing.tile([1, 128], BF16, name="oner", tag="oner")
        mb_sb = sing.tile([128, NCH], F32, name="mb", tag="mb")
        tri_sb = sing.tile([128, 128], BF16, name="tri", tag="tri")
        wq_sb = [sing.tile([128, GF], BF16, name=f"wq{i}", tag=f"wq{i}") for i in range(CCH)]
        wk_sb = [sing.tile([128, GF], BF16, name=f"wk{i}", tag=f"wk{i}") for i in range(CCH)]
        wv_sb = [sing.tile([128, GF], BF16, name=f"wv{i}", tag=f"wv{i}") for i in range(CCH)]
        wp_sb = [sing.tile([128, C], BF16, name=f"wp{i}", tag=f"wp{i}") for i in range(FB)]
        for i in range(CCH):
            nc.scalar.dma_start(out=wq_sb[i][:], in_=wq_d[i * 128:(i + 1) * 128, :])
        # consts ride the ACT ring after wq (they aren't needed until the
        # first bias-add / exp, ~10 us in; ahead of wq they'd stall the
        # first matmul by ~8 us of ring-FIFO latency)
        nc.scalar.dma_start(out=bq_sb[:], in_=bq_d[:])
        nc.scalar.dma_start(out=bk_sb[:], in_=bk_d[:])
        nc.scalar.dma_start(out=bv_sb[:], in_=bv_d[:])
        nc.scalar.dma_start(out=oner_sb[:], in_=oner_d[:])
        nc.scalar.dma_start(out=mb_sb[:], in_=mb_d[:])
        nc.scalar.dma_start(out=tri_sb[:], in_=tri_d[:])
        for i in range(CCH):
            nc.scalar.dma_start(out=wk_sb[i][:], in_=wk_d[i * 128:(i + 1) * 128, :])
        for i in range(CCH):
            nc.scalar.dma_start(out=wv_sb[i][:], in_=wv_d[i * 128:(i + 1) * 128, :])
        for i in range(FB):
            nc.scalar.dma_start(out=wp_sb[i][:], in_=wp_d[i * 128:(i + 1) * 128, :])

        # per-(fb, tb) activation tiles (bf16)
        qt_sb = [[sing.tile([128, TB], BF16, name=f"qt{f}_{t}", tag=f"qt{f}_{t}")
                  for t in range(NTB)] for f in range(FB)]
        kt_sb = [[sing.tile([128, TB], BF16, name=f"kt{f}_{t}", tag=f"kt{f}_{t}")
                  for t in range(NTB)] for f in range(FB)]
        ot_sb = [[sing.tile([128, TB], BF16, name=f"ot{f}_{t}", tag=f"ot{f}_{t}")
                  for t in range(NTB)] for f in range(FB)]
        v_sb = [sing.tile([128, HPG, HD + 1], BF16, name=f"v{i}", tag=f"v{i}")
                for i in range(NCH)]
        for i in range(NCH):
            nc.gpsimd.memset(v_sb[i][:, :, HD:HD + 1], 1.0)

        def qkv(tb):
            xts = []
            for cc in range(CCH):
                xt_t = xstream.tile([128, TB], BF16, name="xts", tag="xts")
                nc.sync.dma_start(
                    out=xt_t[:], in_=xt_d[cc * 128:(cc + 1) * 128,
                                          tb * TB:(tb + 1) * TB])
                xts.append(xt_t)
            for fb in range(FB):
                pq = ps_a.tile([128, TB], mybir.dt.float32, name="pq", tag="pa")
                for cc in range(CCH):
                    nc.tensor.matmul(pq[:], lhsT=wq_sb[cc][:, fb * 128:(fb + 1) * 128],
                                     rhs=xts[cc][:], start=(cc == 0), stop=(cc == CCH - 1))
                nc.vector.tensor_scalar_add(qt_sb[fb][tb][:], pq[:], bq_sb[:, fb:fb + 1])
            for fb in range(FB):
                pk = ps_a.tile([128, TB], mybir.dt.float32, name="pk", tag="pa")
                for cc in range(CCH):
                    nc.tensor.matmul(pk[:], lhsT=wk_sb[cc][:, fb * 128:(fb + 1) * 128],
                                     rhs=xts[cc][:], start=(cc == 0), stop=(cc == CCH - 1))
                nc.vector.tensor_scalar_add(kt_sb[fb][tb][:], pk[:], bk_sb[:, fb:fb + 1])
            for tt in range(4):
                pv = ps_a.tile([128, GF], mybir.dt.float32, name="pv", tag="pa")
                for cc in range(CCH):
                    nc.tensor.matmul(pv[:], lhsT=xts[cc][:, tt * 128:(tt + 1) * 128],
                                     rhs=wv_sb[cc][:], start=(cc == 0), stop=False)
                nc.tensor.matmul(pv[:], lhsT=oner_sb[:], rhs=bv_sb[:],
                                 start=False, stop=True)
                nc.vector.tensor_copy(
                    v_sb[tb * 4 + tt][:, :, 0:HD],
                    pv[:].rearrange("p (h d) -> p h d", h=HPG))

        def attn(qb, hp):
            # head PAIR hp: the two K=64 logit matmuls sit at partition
            # offsets 0/64 and run concurrently in the PE array (row tiling).
            n_kc = 4 * qb + 4
            fb = hp
            h0, h1 = 2 * hp, 2 * hp + 1
            pso0 = ps_o.tile([HD + 1, TB], mybir.dt.float32, name="pso0", tag="pso0")
            pso1 = ps_o.tile([HD + 1, TB], mybir.dt.float32, name="pso1", tag="pso1")
            for kc in range(n_kc):
                j = kc - 4 * qb
                qoff = max(j, 0) * 128
                width = TB - qoff
                tbk, csl = kc // 4, slice((kc % 4) * 128, (kc % 4 + 1) * 128)
                qsl = slice(qoff, TB)
                pl0 = ps_l.tile([128, TB], mybir.dt.float32, name="pl0", tag="pl0")
                pl1 = ps_l.tile([128, TB], mybir.dt.float32, name="pl1", tag="pl1")
                nc.tensor.matmul(pl0[:, 0:width], lhsT=kt_sb[fb][tbk][0:64, csl],
                                 rhs=qt_sb[fb][qb][0:64, qsl], start=True, stop=True)
                nc.tensor.matmul(pl1[:, 0:width], lhsT=kt_sb[fb][tbk][64:128, csl],
                                 rhs=qt_sb[fb][qb][64:128, qsl], start=True, stop=True)
                pt0 = pts.tile([128, TB], BF16, name="pt0", tag="pt0")
                pt1 = pts.tile([128, TB], BF16, name="pt1", tag="pt1")
                nc.scalar.activation(out=pt0[:, 0:width], in_=pl0[:, 0:width],
                                     func=EXP, bias=mb_sb[:, kc:kc + 1], scale=0.125)
                nc.scalar.activation(out=pt1[:, 0:width], in_=pl1[:, 0:width],
                                     func=EXP, bias=mb_sb[:, kc:kc + 1], scale=0.125)
                if j >= 0:
                    nc.vector.tensor_mul(pt0[:, 0:128], pt0[:, 0:128], tri_sb[:])
                    nc.vector.tensor_mul(pt1[:, 0:128], pt1[:, 0:128], tri_sb[:])
                nc.tensor.matmul(pso0[:, qoff:TB], lhsT=v_sb[kc][:, h0, :],
                                 rhs=pt0[:, 0:width],
                                 start=(kc == 0), stop=(kc == n_kc - 1),
                                 skip_group_check=True)
                nc.tensor.matmul(pso1[:, qoff:TB], lhsT=v_sb[kc][:, h1, :],
                                 rhs=pt1[:, 0:width],
                                 start=(kc == 0), stop=(kc == n_kc - 1),
                                 skip_group_check=True)
            # normalize rows 0:64 by reciprocal of row 64 (denominator);
            # [1,TB] -> [64,TB] partition broadcast runs on the Pool engine
            for po, pso in ((0, pso0), (64, pso1)):
                # one fast copy releases the PSUM accumulation bank
                ou = misc.tile([HD + 1, TB], mybir.dt.float32, name="ou", tag="ou")
                nc.vector.tensor_copy(ou[:], pso[:])
                r = misc.tile([1, TB], mybir.dt.float32, name="r", tag="r")
                nc.vector.reciprocal(r[:], ou[HD:HD + 1, :])
                rb = rbp.tile([64, TB], mybir.dt.float32, name="rb", tag="rb")
                nc.gpsimd.partition_broadcast(rb[:], r[:])
                nc.vector.tensor_mul(ot_sb[fb][qb][po:po + 64, :],
                                     ou[0:HD, :], rb[:])

        def proj(qb):
            for tt in range(4 * qb, 4 * qb + 4):
                for eb in range(2):
                    py = ps_a.tile([128, TB], mybir.dt.float32, name="py", tag="pa")
                    for fc in range(FB):
                        nc.tensor.matmul(
                            py[:],
                            lhsT=ot_sb[fc][qb][:, (tt % 4) * 128:(tt % 4 + 1) * 128],
                            rhs=wp_sb[fc][:, eb * TB:(eb + 1) * TB],
                            start=(fc == 0), stop=(fc == FB - 1))
                    ys = ystage.tile([128, TB], F32, name="ys", tag="ys")
                    nc.vector.tensor_copy(ys[:], py[:])
                    nc.sync.dma_start(out=y_d[tt * 128:(tt + 1) * 128,
                                              eb * TB:(eb + 1) * TB], in_=ys[:])

        for _rep in range(reps):
            for tb in range(NTB):
                qkv(tb)
                for hp in range(HPG // 2):
                    attn(tb, hp)
                    if hp == 0 and tb > 0:
                        proj(tb - 1)
            proj(NTB - 1)

    nc.compile()
    return nc


def _get_nc():
    if "nc" not in _CACHE:
        _CACHE["nc"] = _build()
    return _CACHE["nc"]


def make_in_maps(x, Wq, bq, Wk, bk, Wv, bv, Wp, bp, attention_mask):
    import ml_dtypes
    BF = ml_dtypes.bfloat16
    x = np.asarray(x, np.float32)
    Wq = np.asarray(Wq, np.float32)
    Wk = np.asarray(Wk, np.float32)
    Wv = np.asarray(Wv, np.float32)
    Wp = np.asarray(Wp, np.float32)
    bq = np.asarray(bq, np.float32)
    bk = np.asarray(bk, np.float32)
    bv = np.asarray(bv, np.float32)
    mask = np.asarray(attention_mask)

    tri = np.triu(np.ones((128, 128), np.float32))  # tri[k, q] = 1 if k <= q
    oner = np.ones((1, 128), np.float32)

    in_maps = []
    for c in range(N_CORES):
        b, hg = divmod(c, 2)
        fsl = slice(hg * GF, (hg + 1) * GF)
        mb = np.where(mask[b] == 0, np.float32(-1e9), np.float32(0.0))
        in_maps.append({
            "xt": np.ascontiguousarray(x[b].T).astype(BF),
            "wq": np.ascontiguousarray(Wq[fsl, :].T).astype(BF),
            "wk": np.ascontiguousarray(Wk[fsl, :].T).astype(BF),
            "wv": np.ascontiguousarray(Wv[fsl, :].T).astype(BF),
            "wp": np.ascontiguousarray(Wp[:, fsl].T).astype(BF),
            "bq": np.ascontiguousarray(bq[fsl].reshape(FB, 128).T),
            "bk": np.ascontiguousarray(bk[fsl].reshape(FB, 128).T),
            "bv": np.ascontiguousarray(bv[fsl].reshape(1, GF)).astype(BF),
            "oner": oner.astype(BF),
            "mb": np.ascontiguousarray(mb.reshape(NCH, 128).T.astype(np.float32)),
            "tri": tri.astype(BF),
        })
    return in_maps


def combine(results, bp):
    bp = np.asarray(bp, np.float32)
    y = np.empty((B, T, C), np.float32)
    for b in range(B):
        y[b] = results[2 * b]["y"] + results[2 * b + 1]["y"] + bp[None, :]
    return y


def kernel(x, Wq, bq, Wk, bk, Wv, bv, Wp, bp, attention_mask):
    from concourse.bass_utils import run_bass_kernel_spmd
    nc = _get_nc()
    in_maps = make_in_maps(x, Wq, bq, Wk, bk, Wv, bv, Wp, bp, attention_mask)
    res = run_bass_kernel_spmd(nc, in_maps, list(range(N_CORES)))
    return combine(res.results, bp)


# revision 6
# speedup vs baseline: 13.5361x; 1.1405x over previous
"""Causal self-attention Bass kernel for Trainium2, SPMD over 8 NeuronCores.

Problem: B=4, T=2048, C=1024, H=16, HD=64 (fp32 in/out).
Sharding: (batch, head-group) — core c handles batch b=c//2 and head group
hg=c%2 (8 heads = 512 features). Each core computes its partial output
projection y_part = O_g @ Wp_g^T; the host sums the two partials per batch
and adds bp.

v2: software-pipelined phases + bf16 operands.
  - All matmul operands are bf16 (PSUM accumulation stays fp32), halving
    DMA traffic, SBUF footprint and enabling FWL weight loads.
  - Emission is tb/qb-major: QKV(tb) -> attention(qb=tb) -> proj(qb-1)
    interleaved, so PE fills the gaps of the ACT(exp)-bound attention
    stretches with projection matmuls (engines execute in program order).
  - qt/kt/ot live as per-(fb, tb) tiles so the Tile dependency tracker
    never serializes a consumer on a whole [128, T] tensor.
  - Weights stream on the Activation HWDGE ring, x/y on the SP ring; the
    first QK matmul only waits for wq+xt (~4 us instead of ~35 us).
  - Softmax denominator broadcast [1,TB]->[64,TB] runs on the idle
    GpSimd/Pool engine (partition_broadcast) instead of a DRAM DMA bounce.
  - logits^T layout [k, q]: softmax along the PARTITION axis k; no max
    subtraction (logits bounded); key padding mask enters as the Exp
    activation's per-partition bias; causal diagonal via tri mask multiply.
  - V tiles carry a ones column so P@V_aug also yields the denominator.
"""

import numpy as np

B, T, C, H = 4, 2048, 1024, 16
HD = C // H            # 64
N_CORES = 8
GF = 512               # features per head group
HPG = 8                # heads per group
TB = 512               # t-block width
NTB = T // TB          # 4
NCH = T // 128         # 16 key chunks / token tiles
FB = GF // 128         # 4 feature blocks per group
CCH = C // 128         # 8 contraction chunks

_CACHE = {}


def _build(reps=1):
    import contextlib
    import concourse.bass as bass
    import concourse.tile as tile
    import concourse.mybir as mybir
    from concourse import bacc

    F32 = mybir.dt.float32
    BF16 = mybir.dt.bfloat16
    EXP = mybir.ActivationFunctionType.Exp

    nc = bacc.Bacc("TRN2", target_bir_lowering=False, debug=False,
                   num_devices=N_CORES)

    xt_d = nc.dram_tensor("xt", [C, T], BF16, kind="ExternalInput")
    wq_d = nc.dram_tensor("wq", [C, GF], BF16, kind="ExternalInput")
    wk_d = nc.dram_tensor("wk", [C, GF], BF16, kind="ExternalInput")
    wv_d = nc.dram_tensor("wv", [C, GF], BF16, kind="ExternalInput")
    wp_d = nc.dram_tensor("wp", [GF, C], BF16, kind="ExternalInput")
    bq_d = nc.dram_tensor("bq", [128, FB], F32, kind="ExternalInput")
    bk_d = nc.dram_tensor("bk", [128, FB], F32, kind="ExternalInput")
    bv_d = nc.dram_tensor("bv", [1, GF], BF16, kind="ExternalInput")
    oner_d = nc.dram_tensor("oner", [1, 128], BF16, kind="ExternalInput")
    mb_d = nc.dram_tensor("mb", [128, NCH], F32, kind="ExternalInput")
    tri_d = nc.dram_tensor("tri", [128, 128], BF16, kind="ExternalInput")
    y_d = nc.dram_tensor("y", [T, C], F32, kind="ExternalOutput")

    with tile.TileContext(nc) as tc, contextlib.ExitStack() as ctx:
        sing = ctx.enter_context(tc.tile_pool(name="sing", bufs=1))
        xstream = ctx.enter_context(tc.tile_pool(name="xstream", bufs=10))
        pts = ctx.enter_context(tc.tile_pool(name="pts", bufs=3))
        misc = ctx.enter_context(tc.tile_pool(name="misc", bufs=2))
        rbp = ctx.enter_context(tc.tile_pool(name="rbp", bufs=2))
        ystage = ctx.enter_context(tc.tile_pool(name="ystage", bufs=3))
        ps_a = ctx.enter_context(tc.tile_pool(name="ps_a", bufs=2, space="PSUM"))
        ps_l = ctx.enter_context(tc.tile_pool(name="ps_l", bufs=2, space="PSUM"))
        ps_o = ctx.enter_context(tc.tile_pool(name="ps_o", bufs=1, space="PSUM"))

        # constants + weights on the ACT HWDGE ring; x/y on the SP ring
        bq_sb = sing.tile([128, FB], F32, name="bq", tag="bq")
        bk_sb = sing.tile([128, FB], F32, name="bk", tag="bk")
        bv_sb = sing.tile([1, GF], BF16, name="bv", tag="bv")
        oner_sb = sing.tile([1, 128], BF16, name="oner", tag="oner")
        mb_sb = sing.tile([128, NCH], F32, name="mb", tag="mb")
        tri_sb = sing.tile([128, 128], BF16, name="tri", tag="tri")
        wq_sb = [sing.tile([128, GF], BF16, name=f"wq{i}", tag=f"wq{i}") for i in range(CCH)]
        wk_sb = [sing.tile([128, GF], BF16, name=f"wk{i}", tag=f"wk{i}") for i in range(CCH)]
        wv_sb = [sing.tile([128, GF], BF16, name=f"wv{i}", tag=f"wv{i}") for i in range(CCH)]
        wp_sb = [sing.tile([128, C], BF16, name=f"wp{i}", tag=f"wp{i}") for i in range(FB)]
        for i in range(CCH):
            nc.scalar.dma_start(out=wq_sb[i][:], in_=wq_d[i * 128:(i + 1) * 128, :])
        # consts ride the ACT ring after wq (they aren't needed until the
        # first bias-add / exp, ~10 us in; ahead of wq they'd stall the
        # first matmul by ~8 us of ring-FIFO latency)
        nc.scalar.dma_start(out=bq_sb[:], in_=bq_d[:])
        nc.scalar.dma_start(out=bk_sb[:], in_=bk_d[:])
        nc.scalar.dma_start(out=bv_sb[:], in_=bv_d[:])
        nc.scalar.dma_start(out=oner_sb[:], in_=oner_d[:])
        nc.scalar.dma_start(out=mb_sb[:], in_=mb_d[:])
        nc.scalar.dma_start(out=tri_sb[:], in_=tri_d[:])
        for i in range(CCH):
            nc.scalar.dma_start(out=wk_sb[i][:], in_=wk_d[i * 128:(i + 1) * 128, :])
        for i in range(CCH):
            nc.scalar.dma_start(out=wv_sb[i][:], in_=wv_d[i * 128:(i + 1) * 128, :])
        for i in range(FB):
            nc.scalar.dma_start(out=wp_sb[i][:], in_=wp_d[i * 128:(i + 1) * 128, :])

        # per-(fb, tb) activation tiles (bf16)
        qt_sb = [[sing.tile([128, TB], BF16, name=f"qt{f}_{t}", tag=f"qt{f}_{t}")
                  for t in range(NTB)] for f in range(FB)]
        kt_sb = [[sing.tile([128, TB], BF16, name=f"kt{f}_{t}", tag=f"kt{f}_{t}")
                  for t in range(NTB)] for f in range(FB)]
        ot_sb = [[sing.tile([128, TB], BF16, name=f"ot{f}_{t}", tag=f"ot{f}_{t}")
                  for t in range(NTB)] for f in range(FB)]
        v_sb = [sing.tile([128, HPG, HD + 1], BF16, name=f"v{i}", tag=f"v{i}")
                for i in range(NCH)]
        for i in range(NCH):
            nc.gpsimd.memset(v_sb[i][:, :, HD:HD + 1], 1.0)

        def qkv(tb):
            xts = []
            for cc in range(CCH):
                xt_t = xstream.tile([128, TB], BF16, name="xts", tag="xts")
                nc.sync.dma_start(
                    out=xt_t[:], in_=xt_d[cc * 128:(cc + 1) * 128,
                                          tb * TB:(tb + 1) * TB])
                xts.append(xt_t)
            for fb in range(FB):
                pq = ps_a.tile([128, TB], mybir.dt.float32, name="pq", tag="pa")
                for cc in range(CCH):
                    nc.tensor.matmul(pq[:], lhsT=wq_sb[cc][:, fb * 128:(fb + 1) * 128],
                                     rhs=xts[cc][:], start=(cc == 0), stop=(cc == CCH - 1))
                nc.vector.tensor_scalar_add(qt_sb[fb][tb][:], pq[:], bq_sb[:, fb:fb + 1])
            for fb in range(FB):
                pk = ps_a.tile([128, TB], mybir.dt.float32, name="pk", tag="pa")
                for cc in range(CCH):
                    nc.tensor.matmul(pk[:], lhsT=wk_sb[cc][:, fb * 128:(fb + 1) * 128],
                                     rhs=xts[cc][:], start=(cc == 0), stop=(cc == CCH - 1))
                nc.vector.tensor_scalar_add(kt_sb[fb][tb][:], pk[:], bk_sb[:, fb:fb + 1])
            for tt in range(4):
                pv = ps_a.tile([128, GF], mybir.dt.float32, name="pv", tag="pa")
                for cc in range(CCH):
                    nc.tensor.matmul(pv[:], lhsT=xts[cc][:, tt * 128:(tt + 1) * 128],
                                     rhs=wv_sb[cc][:], start=(cc == 0), stop=False)
                nc.tensor.matmul(pv[:], lhsT=oner_sb[:], rhs=bv_sb[:],
                                 start=False, stop=True)
                nc.vector.tensor_copy(
                    v_sb[tb * 4 + tt][:, :, 0:HD],
                    pv[:].rearrange("p (h d) -> p h d", h=HPG))

        def attn(qb, hp):
            # head PAIR hp: the two K=64 logit matmuls sit at partition
            # offsets 0/64 and run concurrently in the PE array (row tiling).
            n_kc = 4 * qb + 4
            fb = hp
            h0, h1 = 2 * hp, 2 * hp + 1
            pso0 = ps_o.tile([HD + 1, TB], mybir.dt.float32, name="pso0", tag="pso0")
            pso1 = ps_o.tile([HD + 1, TB], mybir.dt.float32, name="pso1", tag="pso1")
            for kc in range(n_kc):
                j = kc - 4 * qb
                qoff = max(j, 0) * 128
                width = TB - qoff
                tbk, csl = kc // 4, slice((kc % 4) * 128, (kc % 4 + 1) * 128)
                qsl = slice(qoff, TB)
                pl0 = ps_l.tile([128, TB], mybir.dt.float32, name="pl0", tag="pl0")
                pl1 = ps_l.tile([128, TB], mybir.dt.float32, name="pl1", tag="pl1")
                nc.tensor.matmul(pl0[:, 0:width], lhsT=kt_sb[fb][tbk][0:64, csl],
                                 rhs=qt_sb[fb][qb][0:64, qsl], start=True, stop=True)
                nc.tensor.matmul(pl1[:, 0:width], lhsT=kt_sb[fb][tbk][64:128, csl],
                                 rhs=qt_sb[fb][qb][64:128, qsl], start=True, stop=True)
                pt0 = pts.tile([128, TB], BF16, name="pt0", tag="pt0")
                pt1 = pts.tile([128, TB], BF16, name="pt1", tag="pt1")
                nc.scalar.activation(out=pt0[:, 0:width], in_=pl0[:, 0:width],
                                     func=EXP, bias=mb_sb[:, kc:kc + 1], scale=0.125)
                nc.scalar.activation(out=pt1[:, 0:width], in_=pl1[:, 0:width],
                                     func=EXP, bias=mb_sb[:, kc:kc + 1], scale=0.125)
                if j >= 0:
                    nc.vector.tensor_mul(pt0[:, 0:128], pt0[:, 0:128], tri_sb[:])
                    nc.vector.tensor_mul(pt1[:, 0:128], pt1[:, 0:128], tri_sb[:])
                nc.tensor.matmul(pso0[:, qoff:TB], lhsT=v_sb[kc][:, h0, :],
                                 rhs=pt0[:, 0:width],
                                 start=(kc == 0), stop=(kc == n_kc - 1),
                                 skip_group_check=True)
                nc.tensor.matmul(pso1[:, qoff:TB], lhsT=v_sb[kc][:, h1, :],
                                 rhs=pt1[:, 0:width],
                                 start=(kc == 0), stop=(kc == n_kc - 1),
                                 skip_group_check=True)
            # normalize rows 0:64 by reciprocal of row 64 (denominator);
            # [1,TB] -> [64,TB] partition broadcast runs on the Pool engine
            for po, pso in ((0, pso0), (64, pso1)):
                # one fast copy releases the PSUM accumulation bank
                ou = misc.tile([HD + 1, TB], mybir.dt.float32, name="ou", tag="ou")
                nc.vector.tensor_copy(ou[:], pso[:])
                r = misc.tile([1, TB], mybir.dt.float32, name="r", tag="r")
                nc.vector.reciprocal(r[:], ou[HD:HD + 1, :])
                rb = rbp.tile([64, TB], mybir.dt.float32, name="rb", tag="rb")
                nc.gpsimd.partition_broadcast(rb[:], r[:])
                nc.vector.tensor_mul(ot_sb[fb][qb][po:po + 64, :],
                                     ou[0:HD, :], rb[:])

        def proj(qb):
            for tt in range(4 * qb, 4 * qb + 4):
                for eb in range(2):
                    py = ps_a.tile([128, TB], mybir.dt.float32, name="py", tag="pa")
                    for fc in range(FB):
                        nc.tensor.matmul(
                            py[:],
                            lhsT=ot_sb[fc][qb][:, (tt % 4) * 128:(tt % 4 + 1) * 128],
                            rhs=wp_sb[fc][:, eb * TB:(eb + 1) * TB],
                            start=(fc == 0), stop=(fc == FB - 1))
                    ys = ystage.tile([128, TB], F32, name="ys", tag="ys")
                    nc.vector.tensor_copy(ys[:], py[:])
                    nc.sync.dma_start(out=y_d[tt * 128:(tt + 1) * 128,
                                              eb * TB:(eb + 1) * TB], in_=ys[:])

        for _rep in range(reps):
            for tb in range(NTB):
                qkv(tb)
                for hp in range(HPG // 2):
                    attn(tb, hp)
                    if hp == 0 and tb > 0:
                        proj(tb - 1)
            proj(NTB - 1)

    nc.compile()
    return nc


def _get_nc():
    if "nc" not in _CACHE:
        _CACHE["nc"] = _build()
    return _CACHE["nc"]


def make_in_maps(x, Wq, bq, Wk, bk, Wv, bv, Wp, bp, attention_mask):
    import ml_dtypes
    BF = ml_dtypes.bfloat16
    x = np.asarray(x, np.float32)
    Wq = np.asarray(Wq, np.float32)
    Wk = np.asarray(Wk, np.float32)
    Wv = np.asarray(Wv, np.float32)
    Wp = np.asarray(Wp, np.float32)
    bq = np.asarray(bq, np.float32)
    bk = np.asarray(bk, np.float32)
    bv = np.asarray(bv, np.float32)
    mask = np.asarray(attention_mask)

    tri = np.triu(np.ones((128, 128), np.float32))  # tri[k, q] = 1 if k <= q
    oner = np.ones((1, 128), np.float32)

    in_maps = []
    for c in range(N_CORES):
        b, hg = divmod(c, 2)
        fsl = slice(hg * GF, (hg + 1) * GF)
        mb = np.where(mask[b] == 0, np.float32(-1e9), np.float32(0.0))
        in_maps.append({
            "xt": np.ascontiguousarray(x[b].T).astype(BF),
            "wq": np.ascontiguousarray(Wq[fsl, :].T).astype(BF),
            "wk": np.ascontiguousarray(Wk[fsl, :].T).astype(BF),
            "wv": np.ascontiguousarray(Wv[fsl, :].T).astype(BF),
            "wp": np.ascontiguousarray(Wp[:, fsl].T).astype(BF),
            "bq": np.ascontiguousarray(bq[fsl].reshape(FB, 128).T),
            "bk": np.ascontiguousarray(bk[fsl].reshape(FB, 128).T),
            "bv": np.ascontiguousarray(bv[fsl].reshape(1, GF)).astype(BF),
            "oner": oner.astype(BF),
            "mb": np.ascontiguousarray(mb.reshape(NCH, 128).T.astype(np.float32)),
            "tri": tri.astype(BF),
        })
    return in_maps


def combine(results, bp):
    bp = np.asarray(bp, np.float32)
    y = np.empty((B, T, C), np.float32)
    for b in range(B):
        y[b] = results[2 * b]["y"] + results[2 * b + 1]["y"] + bp[None, :]
    return y


def kernel(x, Wq, bq, Wk, bk, Wv, bv, Wp, bp, attention_mask):
    from concourse.bass_utils import run_bass_kernel_spmd
    nc = _get_nc()
    in_maps = make_in_maps(x, Wq, bq, Wk, bk, Wv, bv, Wp, bp, attention_mask)
    res = run_bass_kernel_spmd(nc, in_maps, list(range(N_CORES)))
    return combine(res.results, bp)
